# revision 36
# baseline (speedup 1.0000x reference)
"""Trainium2 Bass kernel for nn_AutoregressiveCDF (MADE + rational-quadratic
spline CDF, product over features).  v9.

Strategy: pure data-parallel over 8 NeuronCores (batch 16384 -> 8 x 2048).
Per core, row-block pipelined (4 blocks x 512 rows):

- MADE hidden units are degree-sorted offline, making the masked H x H
  weights block-upper-triangular at 128 granularity: mm1/mm2 skip 6/16 chunk
  matmuls and the output GEMM contracts only K = q+1 hidden chunks for
  feature quarter q.  GEMMs in bf16, PSUM fp32.
- Spline per 128-row chunk around NORMALIZED width/height streams:
  EWn = exp(uw*s)*CF/S so per-feature bin widths (EWn + MIN_BIN) sum to
  exactly 1 (W-side fp32 + accurate reciprocal keep the invariant to ~1e-6;
  bf16 would drift ~3e-3 by feature 63).  A fused custom DVE op
  (U_EDGE_ANT: u = x_chained >= running_cumsum(EWn + MIN_BIN)) emits the
  bin-membership mask in ONE pass -- no materialized edges, no cumsum, no
  tensor-tensor compare.  Cross-feature chaining is exact because each
  feature advances the running edge by exactly 1.0; the compare side is
  x + iota(f).
- The six per-(row,feature) gathers are chained "capture" scans (stride-0
  output APs).  W/H scans run on the normalized streams via SCAN_MULADD_ANT
  (state += u*(s + MIN_BIN)), so their captures ARE in_cw/in_ch directly --
  no searchsorted index is ever materialized.  D-side scans run on softplus
  differences one pipeline stage later.
- Engine discipline learned from traces: ANY sustained GpSimd activity
  concurrent with the spline taxes every engine 25-40% (SBUF contention),
  so GpSimd only does the 3 batched residual adds t += v per block (fixed
  ~5us each); everything else stays on DVE/ACT.  The per-engine queues are
  in-order, so the NEXT block's trunk is emitted as 10 staggered steps
  between this block's spline chunks, each step's cross-engine producers
  emitted a slot earlier; ctx transposes for late chunks are likewise
  deferred and their loads prefetched.  Large weight DMAs are split into
  pieces so they round-robin across DMA queues (a monolithic 1.5MB blob
  serializes ~67us on one queue).
"""

import numpy as np
from contextlib import ExitStack

import concourse.bass as bass
import concourse.bacc as bacc
import concourse.tile as tile
from concourse import mybir
from concourse.bass_utils import run_bass_kernel_spmd

F32 = mybir.dt.float32
BF16 = mybir.dt.bfloat16

# problem sizes (hardcoded per contract)
B, F, H, C = 16384, 64, 512, 512
NB = 30
MULT = 3 * NB + 1            # 91
NBLOCKS = 3
NCORES = 8
MIN_BIN = 1e-3
MIN_DERIV = 1e-3
CF = float(1.0 - MIN_BIN * NB)
SCALE = float(np.float32(1.0 / np.sqrt(H)))
KH = H // 128                 # 4 hidden chunks
NQ = 4                        # feature quarters
FQ = F // NQ                  # 16 features per quarter
WQ = FQ * NB                  # 480 w/h cols per quarter
DQ = FQ * (NB + 1)            # 496 d cols per quarter
QCOLS = 2 * WQ + DQ           # 1456 cols per quarter
PAIRS = ((0, 1), (2, 3))      # psum pairing: low quarters first (wo0/wo1 arrive earliest)

# knobs
TRACE = False
MM_DT = BF16                  # kept for test.py compat
LAST_RESULTS = None
DBG = None                    # debug-dump tensor name (see _build dbg_shapes)

_CACHE = {}


def _masks():
    d_in = np.arange(1, F + 1)
    d_h = np.arange(H) % max(1, F - 1) + min(1, F - 1)
    m_in = (d_h[None, :] >= d_in[:, None]).astype(np.float32)
    m_hh = (d_h[None, :] >= d_h[:, None]).astype(np.float32)
    d_out = np.repeat(d_in, MULT)
    m_out = (d_out[None, :] > d_h[:, None]).astype(np.float32)
    return m_in, m_hh, m_out, d_h


def _scan_mul_ref(in0, in1, s0, s1, imm2):
    a = np.asarray(in0, np.float32).reshape(np.asarray(in0).shape[0], -1)
    b = np.asarray(in1, np.float32).reshape(a.shape)
    return np.cumsum(a * b, axis=1, dtype=np.float32).reshape(
        np.asarray(in0).shape)


def _scan_muladd_ref(in0, in1, s0, s1, imm2):
    a = np.asarray(in0, np.float32).reshape(np.asarray(in0).shape[0], -1)
    b = np.asarray(in1, np.float32).reshape(a.shape)
    return np.cumsum(a * (b + s0), axis=1, dtype=np.float32).reshape(
        np.asarray(in0).shape)


def _u_edge_ref(in0, in1, s0, s1, imm2):
    a = np.asarray(in0, np.float32).reshape(np.asarray(in0).shape[0], -1)
    b = np.asarray(in1, np.float32).reshape(a.shape)
    e = np.cumsum(b + s0, axis=1, dtype=np.float32)
    return (a >= e).astype(np.float32).reshape(np.asarray(in0).shape)


def _register_ops():
    """Custom DVE ops:
    SCAN_MUL_ANT: out = chained_cumsum(in0*in1)  (masked-sum capture scans)
    U_EDGE_ANT:   out = (in0 >= chained_cumsum(in1 + s0))  (fused edges+cmp)
    """
    import concourse.dve_ops as dve_ops
    from concourse.dve_spec import Spec, Src0, Src1, C0, scan, AluOp, lower
    from concourse.dve_uop import DveOpSpec
    have = {op.name: op for op in dve_ops.OPS}
    specs = (
        ("SCAN_MUL_ANT", Spec(body=scan(AluOp.ADD, Src0 * Src1),
                              reference=_scan_mul_ref), True),
        ("SCAN_MULADD_ANT", Spec(body=scan(AluOp.ADD, Src0 * (Src1 + C0)),
                                 reference=_scan_muladd_ref), True),
        ("U_EDGE_ANT", Spec(body=Src0 >= scan(AluOp.ADD, Src1 + C0),
                            reference=_u_edge_ref), True),
    )
    out = []
    row = max(dve_ops._SUB_OPCODE_FOR_NAME.values()) + 1
    for name, spec, rd1 in specs:
        if name in have:
            out.append(have[name])
            continue
        assert row < 0x20
        shas = {}
        for ver in ("v3", "v4"):
            u = lower(spec, ver=ver)
            shas[ver] = DveOpSpec(name=name, opcode=row, uops=u,
                                  rd1_en=rd1).sha(ver)
        op = dve_ops.DveOp(name, spec, subdim=False, uops_sha=shas)
        dve_ops.OPS.append(op)
        dve_ops.CUSTOM_DVE_SPECS[name] = spec
        dve_ops._SUB_OPCODE_FOR_NAME[name] = row
        out.append(op)
        row += 1
    return out


class _Bacc(bacc.Bacc):
    """Bacc with a trimmed activation-table list so Exp and Ln share one
    table and Sigmoid another (no per-chunk ACT_TABLE_LOAD thrash)."""

    _KEEP_TABLES = ("natural_log_exp_and_others", "sigmoid_and_others")

    def insert_act_table_loads(self):
        import bass_rust as _bass_rust
        from concourse.hw_specs import get_activation_tables
        import concourse.mybir as _mb
        has_activation = any(
            isinstance(i, _mb.InstActivation)
            for b in self.main_func.blocks
            for i in b.instructions
        )
        if not has_activation:
            return
        all_tables = get_activation_tables(self.m.arch)
        tables = [(k, (v if k in self._KEEP_TABLES else set()))
                  for k, v in all_tables.items()]
        _bass_rust.insert_act_table_loads(self, tables)


def _build(bc, dbg=None):
    nch = bc // 128               # 16 row chunks of 128
    NBLK = 4
    BSW = bc // NBLK              # 512 rows per block
    GPB = BSW // 128              # 4 chunks per block
    scan_mul, scan_muladd, u_edge = _register_ops()
    nc = _Bacc("TRN2", target_bir_lowering=False, debug=False,
               enable_asserts=False)

    def din(name, shape, dt=F32):
        return nc.dram_tensor(name, list(shape), dt, kind="ExternalInput").ap()

    dbg_shapes = {
        "tbf": ([NBLK, 128, KH, bc // NBLK], BF16),
        "EW": ([bc // 128, 128, F, NB], F32),
        "EH": ([bc // 128, 128, F, NB], F32),
        "D": ([bc // 128, 128, F, NB + 1], BF16),
        "u": ([bc // 128, 128, F, NB], BF16),
        "cap": ([NBLK, 128, bc // NBLK // 128, 6, F], F32),
        "sm": ([NBLK, 128, 2, bc // NBLK // 128, F], F32),
        "xpc": ([bc // 128, 128, F], F32),
    }
    dbg_d = None
    if dbg is not None:
        shp, ddt = dbg_shapes[dbg]
        dbg_d = nc.dram_tensor("dbg", list(shp), ddt,
                               kind="ExternalOutput").ap()

    pred = din("pred", (bc, F))
    ctxm = din("ctx", (bc, C))
    win = din("win", (F, H), BF16)
    wc4 = din("wc4", (C, (NBLOCKS + 1) * H), BF16)
    wb1 = din("wb1", (NBLOCKS, H, H), BF16)
    wb2 = din("wb2", (NBLOCKS, H, H), BF16)
    wo_d = [din(f"wo{k}", (128, (NQ - k) * QCOLS), BF16) for k in range(KH)]
    b1 = din("b1", (H,))
    bb1 = din("bb1", (NBLOCKS, H))
    bb2 = din("bb2", (NBLOCKS, H))
    bcb = din("bcb", (NBLOCKS, H))
    ident = din("ident", (128, 128))
    iotaf = din("iotaf", (F,))
    out_d = nc.dram_tensor("out", [bc], F32, kind="ExternalOutput").ap()

    AX = mybir.AxisListType
    OP = mybir.AluOpType
    ACTF = mybir.ActivationFunctionType

    def bcast(apx, n):
        """AP -> same with a stride-0 inner dim of size n appended."""
        return bass.AP(tensor=apx.tensor, offset=apx.offset,
                       ap=list(apx.ap) + [[0, n]])

    def pbcast(ap1d, p):
        return bass.AP(tensor=ap1d.tensor, offset=ap1d.offset,
                       ap=[[0, p]] + list(ap1d.ap))

    with tile.TileContext(nc) as tc, ExitStack() as ctx:
        const = ctx.enter_context(tc.tile_pool(name="const", bufs=1))
        wp = ctx.enter_context(tc.tile_pool(name="wp", bufs=1))
        persist = ctx.enter_context(tc.tile_pool(name="persist", bufs=1))

        ident_t = const.tile([128, 128], F32)
        nc.sync.dma_start(out=ident_t[:], in_=ident)
        one_t = const.tile([128, 1], F32)
        nc.vector.memset(one_t[:], 1.0)
        mb_t = const.tile([128, 1], F32)
        nc.vector.memset(mb_t[:], MIN_BIN)
        md_t = const.tile([128, 1], F32)
        nc.vector.memset(md_t[:], MIN_DERIV)
        iota_t = const.tile([128, F], F32)
        nc.sync.dma_start(out=iota_t[:], in_=pbcast(iotaf, 128))

        # ---- persistent activations ----
        # ctx_T split in two so deferred in-loop transposes (chunks 8-15)
        # never write a tile the earlier blocks' trunk is reading.
        ctx_Ta = [persist.tile([128, bc // 2], BF16, tag=f"cTa{k}",
                               name=f"cTa{k}") for k in range(KH)]
        ctx_Tb = [persist.tile([128, bc // 2], BF16, tag=f"cTb{k}",
                               name=f"cTb{k}") for k in range(KH)]
        x_T = persist.tile([F, bc], BF16)
        prodb = persist.tile([128, nch], F32)

        def ctxT(k, sl):
            half = bc // 2
            if sl.stop <= half:
                return ctx_Ta[k][:, sl]
            return ctx_Tb[k][:, sl.start - half:sl.stop - half]

        # ---- weights (DMAs ordered so block 0 can start early) ----
        win_t = wp.tile([F, H], BF16)
        wc4_t = [wp.tile([128, (NBLOCKS + 1) * H], BF16, tag=f"wc4_{k}",
                         name=f"wc4_{k}") for k in range(KH)]
        wb1_t = [[wp.tile([128, H - 128 * k], BF16, tag=f"wb1_{i}_{k}",
                          name=f"wb1_{i}_{k}") for k in range(KH)]
                 for i in range(NBLOCKS)]
        wb2_t = [[wp.tile([128, H - 128 * k], BF16, tag=f"wb2_{i}_{k}",
                          name=f"wb2_{i}_{k}") for k in range(KH)]
                 for i in range(NBLOCKS)]
        wo_t = [wp.tile([128, (NQ - k) * QCOLS], BF16, tag=f"wo_{k}",
                        name=f"wo_{k}") for k in range(KH)]
        b1_t = wp.tile([128, KH], F32)
        bb1_t = wp.tile([128, NBLOCKS, KH], F32)
        bb2_t = wp.tile([128, NBLOCKS, KH], F32)
        bcb_t = wp.tile([128, NBLOCKS, KH], F32)

        # in-loop ctx-transpose psum/load pools (alive for the whole kernel)
        pst = ctx.enter_context(tc.tile_pool(name="pst", bufs=2, space="PSUM"))
        pld = ctx.enter_context(tc.tile_pool(name="pld", bufs=4))

        pending_ld = {}

        def ld_ctx(c):
            ld = pld.tile([128, C], F32, tag="ctxld", name="ctxld")
            for hh in range(2):
                nc.sync.dma_start(
                    out=ld[:, hh * (C // 2):(hh + 1) * (C // 2)],
                    in_=ctxm[c * 128:(c + 1) * 128,
                             hh * (C // 2):(hh + 1) * (C // 2)])
            pending_ld[c] = ld

        def tp_ctx(c):
            ld = pending_ld.pop(c, None)
            if ld is None:
                ld_ctx(c)
                ld = pending_ld.pop(c)
            ps = pst.tile([128, KH, 128], F32, tag="tp", name="tp")
            for k in range(KH):
                nc.tensor.transpose(ps[:, k, :], ld[:, k * 128:(k + 1) * 128],
                                    ident_t[:])
            half = bc // 2
            for k in range(KH):
                dstl = ctx_Ta[k] if c * 128 < half else ctx_Tb[k]
                o = c * 128 if c * 128 < half else c * 128 - half
                nc.scalar.activation(out=dstl[:, o:o + 128],
                                     in_=ps[:, k, :], func=ACTF.Copy)

        # prologue: block-0 inputs first (ctx+pred chunks 0-3), then weights
        # in first-use order (wo0 early: the first out-GEMM needs it right
        # after block 0's trunk), remaining transposes last.
        with tc.tile_pool(name="pstp", bufs=2, space="PSUM") as pstp:
            # warm the PE p-state during the initial DMA wait: ident lands
            # within ~1us, everything else takes ~12us to arrive
            for _ in range(10):
                ps = pstp.tile([128, 4, 128], F32, tag="warm", name="warm")
                for w4 in range(4):
                    nc.tensor.matmul(ps[:, w4, :], ident_t[:], ident_t[:],
                                     start=True, stop=True)
            def tp_pred(c):
                pldp = pld.tile([128, F], F32, tag="predld", name="predld")
                nc.sync.dma_start(out=pldp[:],
                                  in_=pred[c * 128:(c + 1) * 128, :])
                ps = pstp.tile([F, 128], F32, tag="tpp", name="tpp")
                nc.tensor.transpose(ps[:], pldp[:], ident_t[:])
                nc.scalar.activation(out=x_T[:, c * 128:(c + 1) * 128],
                                     in_=ps[:], func=ACTF.Copy)

            for c in range(4):
                tp_ctx(c)
                tp_pred(c)
            for c in range(4, nch):
                tp_pred(c)
            nc.sync.dma_start(out=win_t[:], in_=win)
            W4 = (NBLOCKS + 1) * H
            for k in range(KH):
                for qq in range(4):
                    nc.sync.dma_start(
                        out=wc4_t[k][:, qq * (W4 // 4):(qq + 1) * (W4 // 4)],
                        in_=wc4[k * 128:(k + 1) * 128,
                                qq * (W4 // 4):(qq + 1) * (W4 // 4)])
            nc.sync.dma_start(out=b1_t[:], in_=b1.rearrange("(m p) -> p m",
                                                            p=128))
            for tt_, src in ((bb1_t, bb1), (bb2_t, bb2), (bcb_t, bcb)):
                nc.sync.dma_start(out=tt_[:],
                                  in_=src.rearrange("i (m p) -> p i m", p=128))
            def wo_piece(k, qq):
                hw_ = QCOLS // 2
                for hh in range(2):
                    a0 = qq * QCOLS + hh * hw_
                    nc.sync.dma_start(out=wo_t[k][:, a0:a0 + hw_],
                                      in_=wo_d[k][:, a0:a0 + hw_])

            for i in range(NBLOCKS):
                for k in range(KH):
                    ksl = slice(k * 128, (k + 1) * 128)
                    nc.sync.dma_start(out=wb1_t[i][k][:],
                                      in_=wb1[i, ksl, 128 * k:])
                    nc.sync.dma_start(out=wb2_t[i][k][:],
                                      in_=wb2[i, ksl, 128 * k:])
            # ctx 4-7 loads issued now (cheap, early); their transposes are
            # emitted AFTER block 0's trunk so the in-order PE queue never
            # stalls on them before t0
            for c in range(4, 8):
                ld_ctx(c)
            for q in range(NQ):
                for k in range(q + 1):
                    wo_piece(k, q - k)

        TS = nc.vector.tensor_scalar
        TT = nc.vector.tensor_tensor
        STT = nc.vector.scalar_tensor_tensor

        def tscopy(dst, srcap):
            TS(out=dst, in0=srcap, scalar1=0.0, scalar2=None, op0=OP.add)

        def flat3(t):
            return t[:].rearrange("p a b -> p (a b)")

        # ---- main pipeline pools ----
        with tc.tile_pool(name="psa", bufs=2, space="PSUM") as psa, \
             tc.tile_pool(name="psb", bufs=2, space="PSUM") as psb, \
             tc.tile_pool(name="blk", bufs=2) as blkp, \
             tc.tile_pool(name="blk1", bufs=1) as blkp1, \
             tc.tile_pool(name="spl", bufs=2) as spl, \
             tc.tile_pool(name="spl1", bufs=1) as spl1, \
             tc.tile_pool(name="win", bufs=1) as winp, \
             tc.tile_pool(name="chn", bufs=1) as chn:

            tbf_of = {}

            def trunk_steps(blk, dve_add=False):
                """Emit block blk's MADE trunk as 10 staggered closures,
                interleaved between the previous block's spline chunks so
                each op's cross-engine producers land a slot earlier (the
                per-engine queues are in-order: program order = schedule)."""
                bsl = slice(blk * BSW, (blk + 1) * BSW)
                st = {}

                def t0():
                    t_t = blkp.tile([128, KH, BSW], BF16, tag="t", name="t")
                    st["t"] = t_t
                    for m in range(KH):
                        msl = slice(m * 128, (m + 1) * 128)
                        ps = psa.tile([128, BSW], F32, tag="mma", name="mma")
                        nc.tensor.matmul(ps[:], win_t[:, msl], x_T[:, bsl],
                                         start=True, stop=False)
                        for k in range(KH):
                            nc.tensor.matmul(ps[:], wc4_t[k][:, msl],
                                             ctxT(k, bsl),
                                             start=False, stop=(k == KH - 1))
                        nc.scalar.activation(out=t_t[:, m, :], in_=ps[:],
                                             func=ACTF.Identity,
                                             bias=b1_t[:, m:m + 1])

                def gates(ii):
                    if ii == 0:
                        st["g"] = blkp1.tile([128, NBLOCKS, KH, BSW], BF16,
                                             tag="g", name="g")
                    g_t = st["g"]
                    for m in range(KH):
                        csl0 = (ii + 1) * H + m * 128
                        ps = psa.tile([128, BSW], F32, tag="mma", name="mma")
                        for k in range(KH):
                            nc.tensor.matmul(ps[:],
                                             wc4_t[k][:, csl0:csl0 + 128],
                                             ctxT(k, bsl),
                                             start=(k == 0), stop=(k == KH - 1))
                        nc.scalar.activation(out=g_t[:, ii, m, :], in_=ps[:],
                                             func=ACTF.Sigmoid,
                                             bias=bcb_t[:, ii, m:m + 1])

                def rb_a(ii):   # h1 relu + mm1 chains + h2 relus
                    h1 = blkp1.tile([128, KH, BSW], BF16, tag="h1", name="h1")
                    nc.scalar.activation(out=flat3(h1), in_=flat3(st["t"]),
                                         func=ACTF.Relu)
                    h2 = blkp1.tile([128, KH, BSW], BF16, tag="h2", name="h2")
                    st["h2"] = h2
                    for m in range(KH):
                        ps = psa.tile([128, BSW], F32, tag="mma", name="mma")
                        for k in range(m + 1):
                            off = (m - k) * 128
                            nc.tensor.matmul(ps[:],
                                             wb1_t[ii][k][:, off:off + 128],
                                             h1[:, k, :],
                                             start=(k == 0), stop=(k == m))
                        nc.scalar.activation(out=h2[:, m, :], in_=ps[:],
                                             func=ACTF.Relu,
                                             bias=bb1_t[:, ii, m:m + 1])

                def rb_b(ii):   # mm2 chains + gated v = (psum+bb2)*g
                    h2 = st["h2"]
                    v = blkp1.tile([128, KH, BSW], BF16, tag="v", name="v")
                    st["v"] = v
                    for m in range(KH):
                        ps2 = psa.tile([128, BSW], F32, tag="mma", name="mma")
                        for k in range(m + 1):
                            off = (m - k) * 128
                            nc.tensor.matmul(ps2[:],
                                             wb2_t[ii][k][:, off:off + 128],
                                             h2[:, k, :],
                                             start=(k == 0), stop=(k == m))
                        STT(out=v[:, m, :], in0=ps2[:],
                            scalar=bb2_t[:, ii, m:m + 1],
                            in1=st["g"][:, ii, m, :], op0=OP.add, op1=OP.mult)

                def rb_fin(ii):  # t += v (gp flat add; DVE ping-pong when
                    # the DVE is otherwise idle, i.e. block 0 at startup)
                    if dve_add:
                        t2 = blkp.tile([128, KH, BSW], BF16, tag="t", name="t")
                        TT(out=t2[:], in0=st["t"][:], in1=st["v"][:],
                           op=OP.add)
                        st["t"] = t2
                    else:
                        nc.gpsimd.tensor_tensor(out=st["t"][:],
                                                in0=st["t"][:],
                                                in1=st["v"][:], op=OP.add)

                def tbf_cp():
                    # t is bf16: the out-GEMM reads it directly, no copy
                    tbf_of[blk] = st["t"]
                    if dbg == "tbf":
                        nc.sync.dma_start(out=dbg_d[blk], in_=st["t"][:])

                return [
                    t0,                                   # s0 (pre-a0)
                    lambda: gates(0),                     # s1
                    lambda: (gates(1), rb_a(0)),          # s2
                    lambda: (gates(2), rb_b(0)),          # s3
                    lambda: (rb_fin(0), rb_a(1)),         # s4
                    lambda: rb_b(1),                      # s5
                    lambda: (rb_fin(1), rb_a(2)),         # s6
                    lambda: (rb_b(2), rb_fin(2)),         # s7
                    lambda: None,                         # s8
                    tbf_cp,                               # s9 (post-chain)
                ]

            # block 0's trunk runs un-overlapped (nothing to hide it behind)
            for p in trunk_steps(0, dve_add=True):
                p()
            for c in range(4, 8):
                tp_ctx(c)

            for blk in range(NBLK):
                tbf = tbf_of.pop(blk)
                # ---- per-block window buffers ----
                capb = winp.tile([128, GPB, 6, F], F32, tag="capb",
                                 name="capb")
                extb = winp.tile([128, GPB, 4, F], BF16, tag="extb",
                                 name="extb")
                xb = winp.tile([128, GPB, F], F32, tag="xb", name="xb")
                xpcw = winp.tile([128, GPB, F], F32, tag="xpcw", name="xpcw")

                def stage_a(gi):
                    c = blk * GPB + gi
                    csl = slice(c * 128, (c + 1) * 128)
                    gsl = slice(gi * 128, (gi + 1) * 128)
                    nc.sync.dma_start(out=xb[:, gi, :], in_=pred[csl, :])

                    EW = spl1.tile([128, F, NB], BF16, tag="EW", name="EW")
                    EH = spl1.tile([128, F, NB], BF16, tag="EH", name="EH")
                    # Dt is consumed within stage_a (dD + extracts): 1 buf.
                    Dt = spl1.tile([128, F, NB + 1], BF16, tag="Dt", name="Dt")
                    for ty in range(3):       # 0=w 1=h 2=d
                        for pa, pair in enumerate(PAIRS):
                            ncols = DQ if ty == 2 else WQ
                            ps = psb.tile([128, 2, 512], F32, tag="po",
                                          name="po")
                            for si, q in enumerate(pair):
                                for k in range(q + 1):
                                    off = ((q - k) * QCOLS + ty * WQ)
                                    nc.tensor.matmul(
                                        ps[:, si, 0:ncols],
                                        tbf[:, k, gsl],
                                        wo_t[k][:, off:off + ncols],
                                        start=(k == 0), stop=(k == q))
                            qa, qb = pair
                            dst = Dt if ty == 2 else (EW if ty == 0 else EH)
                            dstp = dst[:]
                            out2 = bass.AP(
                                tensor=dstp.tensor,
                                offset=dstp.offset + qa * ncols,
                                ap=[dstp.ap[0], [(qb - qa) * ncols, 2],
                                    [1, ncols]])
                            nc.scalar.activation(
                                out=out2, in_=ps[:, :, 0:ncols],
                                func=ACTF.Exp,
                                scale=(1.0 if ty == 2 else SCALE))
                    # D = ln(exp(ud) + 1) in place
                    nc.scalar.activation(
                        out=Dt[:].rearrange("p f n -> p (f n)"),
                        in_=Dt[:].rearrange("p f n -> p (f n)"),
                        func=ACTF.Ln, bias=one_t[:])
                    # per-feature sums and normalizers.  W-side reciprocal
                    # must be accurate: the chained-edge trick needs
                    # sum(EWn) + NB*MIN_BIN == 1 to ~1e-6.
                    swf = spl1.tile([128, F], F32, tag="swf", name="swf")
                    shf = spl1.tile([128, F], F32, tag="shf", name="shf")
                    nc.vector.tensor_reduce(out=swf[:], in_=EW[:],
                                            axis=AX.X, op=OP.add)
                    nc.vector.tensor_reduce(out=shf[:], in_=EH[:],
                                            axis=AX.X, op=OP.add)
                    rsq = spl1.tile([128, 2, F], F32, tag="rsq", name="rsq")
                    nc.vector.reciprocal(out=rsq[:, 0, :], in_=swf[:])
                    nc.vector.reciprocal_approx_fast(out=rsq[:, 1, :],
                                                     in_=shf[:])
                    crs = spl1.tile([128, 2, F], F32, tag="crs", name="crs")
                    TS(out=flat3(crs), in0=flat3(rsq), scalar1=CF,
                       scalar2=None, op0=OP.mult)
                    # normalized W stream (fp32, for the fused edge compare
                    # only; the masked scans read the raw streams)
                    EWn = spl1.tile([128, F, NB], F32, tag="EWn", name="EWn")
                    TT(out=EWn[:], in0=EW[:],
                       in1=bcast(crs[:, 0, :], NB), op=OP.mult)
                    EHn = spl1.tile([128, F, NB], BF16, tag="EHn", name="EHn")
                    TT(out=EHn[:], in0=EH[:],
                       in1=bcast(crs[:, 1, :], NB), op=OP.mult)
                    # chained compare operand: x + f
                    TT(out=xpcw[:, gi, :], in0=xb[:, gi, :], in1=iota_t[:],
                       op=OP.add)
                    # fused edges+compare: u[j] = [x+f >= e_{j+1}], chained
                    u30 = spl.tile([128, F, NB], BF16, tag="u30", name="u30")
                    nc.vector._custom_dve(u_edge, out=u30[:],
                                          in0=bcast(xpcw[:, gi, :], NB),
                                          in1=EWn[:], s0=MIN_BIN)
                    # W/H masked scans on the normalized streams with the
                    # +MIN_BIN term folded in: captures ARE incw/inch sums
                    for s in range(4):
                        srct = EWn if s < 2 else EHn
                        cap = bass.AP(
                            tensor=capb[:].tensor,
                            offset=capb[:].offset + (gi * 6 + s) * F,
                            ap=[capb[:].ap[0], [1, F], [0, NB - 1]])
                        ssl = bass.AP(tensor=srct[:].tensor,
                                      offset=srct[:].offset + (s % 2),
                                      ap=[srct[:].ap[0], [NB, F], [1, NB - 1]])
                        nc.vector._custom_dve(scan_muladd, out=cap,
                                              in0=u30[:, :, 0:NB - 1],
                                              in1=ssl, s0=MIN_BIN)
                    # softplus differences on DVE: ANY sustained GpSimd
                    # activity during the spline taxes every engine 25-40%
                    # (SBUF contention), so this stays here despite gp slack
                    dD = spl.tile([128, F, NB], BF16, tag="dD", name="dD")
                    TT(out=dD[:], in0=Dt[:, :, 1:NB + 1],
                       in1=Dt[:, :, 0:NB], op=OP.subtract)
                    # extracts: ew0/eh0 normalized (+MIN_BIN folded);
                    # d0/d1 with +MIN_DERIV folded in
                    for j, (src, st_, o, bia) in enumerate((
                            (EWn[:], NB, 0, mb_t[:]), (EHn[:], NB, 0, mb_t[:]),
                            (Dt[:], NB + 1, 0, md_t[:]),
                            (Dt[:], NB + 1, 1, md_t[:]))):
                        src_ap = bass.AP(tensor=src.tensor,
                                         offset=src.offset + o,
                                         ap=[src.ap[0], [st_, F]])
                        if bia is None:
                            nc.scalar.activation(out=extb[:, gi, j, :],
                                                 in_=src_ap, func=ACTF.Copy)
                        else:
                            nc.scalar.activation(out=extb[:, gi, j, :],
                                                 in_=src_ap,
                                                 func=ACTF.Identity, bias=bia)
                    if dbg == "EW":
                        nc.sync.dma_start(out=dbg_d[c], in_=EWn[:])
                    if dbg == "EH":
                        nc.sync.dma_start(out=dbg_d[c], in_=EH[:])
                    if dbg == "D":
                        nc.sync.dma_start(out=dbg_d[c], in_=Dt[:])
                    if dbg == "xpc":
                        nc.sync.dma_start(out=dbg_d[c], in_=xpcw[:, gi, :])
                    if dbg == "u":
                        nc.sync.dma_start(out=dbg_d[c], in_=u30[:])
                    return dD, u30

                def stage_b(gi, tiles):
                    dD, u30 = tiles
                    # D-side masked-sum scans (deferred a stage: dD comes
                    # from GpSimd and must not stall the DVE queue)
                    for s in range(2):
                        cap = bass.AP(
                            tensor=capb[:].tensor,
                            offset=capb[:].offset + (gi * 6 + s + 4) * F,
                            ap=[capb[:].ap[0], [1, F], [0, NB - 1]])
                        dsl = bass.AP(tensor=dD[:].tensor,
                                      offset=dD[:].offset + s,
                                      ap=[dD[:].ap[0], [NB, F], [1, NB - 1]])
                        nc.vector._custom_dve(scan_mul, out=cap,
                                              in0=u30[:, :, 0:NB - 1],
                                              in1=dsl)

                # deferred ctx transposes: chunks 8-15 during blocks 0-1
                tp_sched = {0: [8, 9, 10, 11], 1: [12, 13, 14, 15]}
                tps = tp_sched.get(blk, [])
                steps = trunk_steps(blk + 1) if blk + 1 < NBLK else []

                def filler(sl):
                    if 0 <= sl - 1 < len(tps):
                        tp_ctx(tps[sl - 1])
                    if sl + 1 < len(tps):
                        ld_ctx(tps[sl + 1])
                    if sl < len(steps):
                        steps[sl]()

                # slot 0 (t0 of the next trunk) goes BEFORE stage_a(0): the
                # in-order PE queue must not park it behind this block's
                # first out-GEMM (which waits on tbf/exp consumers).
                for c0 in tps[:2]:
                    ld_ctx(c0)
                filler(0)
                pend = stage_a(0)
                filler(1)
                nxt = stage_a(1)
                filler(2)
                stage_b(0, pend)
                filler(3)
                pend = nxt
                nxt = stage_a(2)
                filler(4)
                stage_b(1, pend)
                filler(5)
                pend = nxt
                nxt = stage_a(3)
                filler(6)
                stage_b(2, pend)
                filler(7)
                stage_b(3, nxt)
                filler(8)

                if dbg == "cap":
                    nc.sync.dma_start(out=dbg_d[blk], in_=capb[:])
                if dbg == "sm":
                    nc.sync.dma_start(out=dbg_d[blk, :, 1], in_=extb[:, :, 0])

                # ---- deferred per-feature chain, [128, GPB*F] tiles ----
                def ct(nm):
                    return chn.tile([128, GPB, F], F32, tag=nm, name=nm)

                def flat(t):
                    return t[:].rearrange("p a f -> p (a f)")

                # un-chain captures: gd[f] = cap[f] - cap[f-1] within each gi
                gd = chn.tile([128, GPB, 6, F], F32, tag="gd", name="gd")
                tscopy(gd[:, :, :, 0:1], capb[:, :, :, 0:1])
                TT(out=gd[:, :, :, 1:F], in0=capb[:, :, :, 1:F],
                   in1=capb[:, :, :, 0:F - 1], op=OP.subtract)
                q1 = gd[:, :, 0, :]
                q2 = gd[:, :, 1, :]
                r1 = gd[:, :, 2, :]
                r2 = gd[:, :, 3, :]
                g5 = gd[:, :, 4, :]
                g6 = gd[:, :, 5, :]
                ew0b = extb[:, :, 0, :]
                eh0b = extb[:, :, 1, :]
                d0b = extb[:, :, 2, :]
                d1b = extb[:, :, 3, :]

                w1 = ct("w1"); w2 = ct("w2"); w3 = ct("w3")
                w6 = ct("w6"); w7 = ct("w7"); w8 = ct("w8")
                w9 = xb  # xb is dead after th; reuse
                # q1 IS incw (scans ran on the normalized stream with +MB);
                # w2 = inw = (q2 - q1) + (EWn[0]+MB) -> rw
                TT(out=w2[:], in0=q2, in1=q1, op=OP.subtract)
                TT(out=w2[:], in0=w2[:], in1=ew0b, op=OP.add)
                nc.vector.reciprocal_approx_fast(out=flat(w2), in_=flat(w2))
                # w3 = th = (x - incw) * rw
                TT(out=w3[:], in0=xb[:], in1=q1, op=OP.subtract)
                TT(out=w3[:], in0=w3[:], in1=w2[:], op=OP.mult)
                # w6 = inh = (r2 - r1) + (EHn[0]+MB)
                TT(out=w6[:], in0=r2, in1=r1, op=OP.subtract)
                TT(out=w6[:], in0=w6[:], in1=eh0b, op=OP.add)
                # w1 = ind ; w8 = s2 = ind + indp1
                TT(out=w1[:], in0=g5, in1=d0b, op=OP.add)
                TT(out=w8[:], in0=g6, in1=d1b, op=OP.add)
                TT(out=w8[:], in0=w8[:], in1=w1[:], op=OP.add)
                # w7 = tt = th*(1-th) ; w9 = th^2
                nc.scalar.activation(out=flat(w7), in_=flat(w3),
                                     func=ACTF.Identity, bias=one_t[:],
                                     scale=-1.0)
                TT(out=w7[:], in0=w3[:], in1=w7[:], op=OP.mult)
                nc.scalar.activation(out=flat(w9), in_=flat(w3),
                                     func=ACTF.Square)
                # w2 = dl = inh * rw (rw dead)
                TT(out=w2[:], in0=w6[:], in1=w2[:], op=OP.mult)
                # w9 = num = inh*(dl*th^2 + ind*tt)
                TT(out=w9[:], in0=w9[:], in1=w2[:], op=OP.mult)
                TT(out=w1[:], in0=w1[:], in1=w7[:], op=OP.mult)
                TT(out=w9[:], in0=w9[:], in1=w1[:], op=OP.add)
                TT(out=w9[:], in0=w9[:], in1=w6[:], op=OP.mult)
                # w8 = den = dl + (s2 - 2dl)*tt -> rden
                STT(out=w8[:], in0=w2[:], scalar=-2.0, in1=w8[:],
                    op0=OP.mult, op1=OP.add)
                TT(out=w8[:], in0=w8[:], in1=w7[:], op=OP.mult)
                TT(out=w8[:], in0=w8[:], in1=w2[:], op=OP.add)
                nc.vector.reciprocal_approx_fast(out=flat(w8), in_=flat(w8))
                sh = r1  # r1 IS inch (normalized captures)
                cdf = w9
                TT(out=cdf[:], in0=cdf[:], in1=w8[:], op=OP.mult)
                TT(out=cdf[:], in0=cdf[:], in1=sh, op=OP.add)
                # product over 64 features
                TT(out=cdf[:, :, 0:32], in0=cdf[:, :, 0:32],
                   in1=cdf[:, :, 32:64], op=OP.mult)
                TT(out=cdf[:, :, 0:16], in0=cdf[:, :, 0:16],
                   in1=cdf[:, :, 16:32], op=OP.mult)
                TT(out=cdf[:, :, 0:8], in0=cdf[:, :, 0:8],
                   in1=cdf[:, :, 8:16], op=OP.mult)
                TT(out=cdf[:, :, 0:4], in0=cdf[:, :, 0:4],
                   in1=cdf[:, :, 4:8], op=OP.mult)
                TT(out=cdf[:, :, 0:2], in0=cdf[:, :, 0:2],
                   in1=cdf[:, :, 2:4], op=OP.mult)
                TT(out=prodb[:, blk * GPB:(blk + 1) * GPB],
                   in0=cdf[:, :, 0:1].rearrange("p a f -> p (a f)"),
                   in1=cdf[:, :, 1:2].rearrange("p a f -> p (a f)"),
                   op=OP.mult)
                # next block's tbf copy: emitted after the chain so the
                # in-order DVE queue never waits on the trunk's last gp add
                filler(9)

            nc.sync.dma_start(out=out_d.rearrange("(c p) -> p c", p=128),
                              in_=prodb[:])

    nc.compile()
    return nc


def _to_bf16(a):
    import ml_dtypes
    return np.ascontiguousarray(np.asarray(a, np.float32)).astype(
        ml_dtypes.bfloat16)


def _prep_shared(W_in, b_in, Wc_in, bc_in, Wb1, bb1, Wb2, bb2, Wcb, bcb,
                 W_out, b_out):
    m_in, m_hh, m_out, d_h = _masks()
    assert not np.any(np.asarray(b_out)), "nonzero b_out not supported"
    perm = np.argsort(d_h, kind="stable")

    W_in = np.asarray(W_in, np.float32) * m_in
    W_in = W_in[:, perm]
    Wc_in = np.asarray(Wc_in, np.float32)[:, perm]
    Wcb = np.asarray(Wcb, np.float32)[:, :, perm]
    Wb1p = (np.asarray(Wb1, np.float32) * m_hh[None])[:, perm][:, :, perm]
    Wb2p = (np.asarray(Wb2, np.float32) * m_hh[None])[:, perm][:, :, perm]
    W_outp = (np.asarray(W_out, np.float32) * m_out)[perm, :]
    b1 = (np.asarray(b_in, np.float32) + np.asarray(bc_in, np.float32))[perm]
    bb1p = np.asarray(bb1, np.float32)[:, perm]
    bb2p = np.asarray(bb2, np.float32)[:, perm]
    bcbp = np.asarray(bcb, np.float32)[:, perm]

    wc4 = np.concatenate([Wc_in, Wcb[0], Wcb[1], Wcb[2]], axis=1)

    # packed W_out blobs: chunk k holds quarters q>=k as [w(480)|h(480)|d(496)]
    wo = []
    for k in range(KH):
        cols = []
        for q in range(k, NQ):
            for ty in range(3):
                for f in range(FQ * q, FQ * (q + 1)):
                    if ty == 0:
                        cols.extend(range(f * MULT, f * MULT + NB))
                    elif ty == 1:
                        cols.extend(range(f * MULT + NB, f * MULT + 2 * NB))
                    else:
                        cols.extend(range(f * MULT + 2 * NB, (f + 1) * MULT))
        blob = W_outp[k * 128:(k + 1) * 128][:, cols]
        assert blob.shape == (128, (NQ - k) * QCOLS)
        wo.append(_to_bf16(blob))

    shared = {
        "win": _to_bf16(W_in),
        "wc4": _to_bf16(wc4),
        "wb1": _to_bf16(Wb1p),
        "wb2": _to_bf16(Wb2p),
        "b1": np.ascontiguousarray(b1),
        "bb1": np.ascontiguousarray(bb1p),
        "bb2": np.ascontiguousarray(bb2p),
        "bcb": np.ascontiguousarray(bcbp),
        "ident": np.eye(128, dtype=np.float32),
        "iotaf": np.arange(F, dtype=np.float32),
    }
    for k in range(KH):
        shared[f"wo{k}"] = wo[k]
    return shared


def kernel(predicates, contexts, W_in, b_in, Wc_in, bc_in, Wb1, bb1, Wb2, bb2,
           Wcb, bcb, W_out, b_out):
    global LAST_RESULTS
    predicates = np.asarray(predicates, dtype=np.float32)
    contexts = np.asarray(contexts, dtype=np.float32)
    bc = predicates.shape[0] // NCORES
    key = (bc, DBG)
    if key not in _CACHE:
        _CACHE[key] = _build(bc, dbg=DBG)
    nc = _CACHE[key]
    shared = _prep_shared(W_in, b_in, Wc_in, bc_in, Wb1, bb1, Wb2, bb2,
                          Wcb, bcb, W_out, b_out)
    in_maps = []
    for cid in range(NCORES):
        sl = slice(cid * bc, (cid + 1) * bc)
        m = dict(shared)
        m["pred"] = np.ascontiguousarray(predicates[sl])
        m["ctx"] = np.ascontiguousarray(contexts[sl])
        in_maps.append(m)
    res = run_bass_kernel_spmd(nc, in_maps, core_ids=list(range(NCORES)),
                               trace=TRACE)
    LAST_RESULTS = res
    return np.concatenate([res.results[i]["out"] for i in range(NCORES)])


# revision 37
# speedup vs baseline: 1.0114x; 1.0114x over previous
"""Trainium2 Bass kernel for nn_AutoregressiveCDF (MADE + rational-quadratic
spline CDF, product over features).  v11.

Strategy: pure data-parallel over 8 NeuronCores (batch 16384 -> 8 x 2048).
Per core, row-block pipelined (4 blocks x 512 rows):

- MADE hidden units are degree-sorted offline, making the masked H x H
  weights block-upper-triangular at 128 granularity: mm1/mm2 skip 6/16 chunk
  matmuls and the output GEMM contracts only K = q+1 hidden chunks for
  feature quarter q.  GEMMs in bf16, PSUM fp32.
- Spline per 128-row chunk around NORMALIZED width/height streams:
  EWn = exp(uw*s)*CF/S so per-feature bin widths (EWn + MIN_BIN) sum to
  exactly 1 (W-side fp32 + accurate reciprocal keep the invariant to ~1e-6;
  bf16 would drift ~3e-3 by feature 63).  A fused custom DVE op
  (U_EDGE_ANT: u = x_chained >= running_cumsum(EWn + MIN_BIN)) emits the
  bin-membership mask in ONE pass -- no materialized edges, no cumsum, no
  tensor-tensor compare.  Cross-feature chaining is exact because each
  feature advances the running edge by exactly 1.0; the compare side is
  x + iota(f).
- The six per-(row,feature) gathers are chained "capture" scans (stride-0
  output APs).  W/H scans run on the normalized streams via SCAN_MULADD_ANT
  (state += u*(s + MIN_BIN)), so their captures ARE in_cw/in_ch directly --
  no searchsorted index is ever materialized.  D-side scans run on softplus
  differences one pipeline stage later.
- The residual accumulator t is bf16 end-to-end: the out-GEMM reads it
  directly (no tbf copy), GpSimd adds are uniform-dtype (mixed f32+bf16 gp
  ops run 2.4x slower), and trunk SBUF traffic halves.
- Engine discipline learned from traces: ANY sustained GpSimd activity
  concurrent with the spline taxes every engine 25-40% (SBUF contention),
  so GpSimd only does the 3 batched residual adds t += v per block (fixed
  ~5us each); everything else stays on DVE/ACT.  The per-engine queues are
  in-order, so the NEXT block's trunk is emitted as 10 staggered steps
  between this block's spline chunks, each step's cross-engine producers
  emitted a slot earlier; ctx transposes for late chunks are likewise
  deferred and their loads prefetched.  Large weight DMAs are split into
  pieces so they round-robin across DMA queues (a monolithic 1.5MB blob
  serializes ~67us on one queue).
"""

import numpy as np
from contextlib import ExitStack

import concourse.bass as bass
import concourse.bacc as bacc
import concourse.tile as tile
from concourse import mybir
from concourse.bass_utils import run_bass_kernel_spmd

F32 = mybir.dt.float32
BF16 = mybir.dt.bfloat16

# problem sizes (hardcoded per contract)
B, F, H, C = 16384, 64, 512, 512
NB = 30
MULT = 3 * NB + 1            # 91
NBLOCKS = 3
NCORES = 8
MIN_BIN = 1e-3
MIN_DERIV = 1e-3
CF = float(1.0 - MIN_BIN * NB)
SCALE = float(np.float32(1.0 / np.sqrt(H)))
KH = H // 128                 # 4 hidden chunks
NQ = 4                        # feature quarters
FQ = F // NQ                  # 16 features per quarter
WQ = FQ * NB                  # 480 w/h cols per quarter
DQ = FQ * (NB + 1)            # 496 d cols per quarter
QCOLS = 2 * WQ + DQ           # 1456 cols per quarter
PAIRS = ((0, 1), (2, 3))      # psum pairing: low quarters first (wo0/wo1 arrive earliest)

# knobs
TRACE = False
MM_DT = BF16                  # kept for test.py compat
LAST_RESULTS = None
DBG = None                    # debug-dump tensor name (see _build dbg_shapes)

_CACHE = {}


def _masks():
    d_in = np.arange(1, F + 1)
    d_h = np.arange(H) % max(1, F - 1) + min(1, F - 1)
    m_in = (d_h[None, :] >= d_in[:, None]).astype(np.float32)
    m_hh = (d_h[None, :] >= d_h[:, None]).astype(np.float32)
    d_out = np.repeat(d_in, MULT)
    m_out = (d_out[None, :] > d_h[:, None]).astype(np.float32)
    return m_in, m_hh, m_out, d_h


def _scan_mul_ref(in0, in1, s0, s1, imm2):
    a = np.asarray(in0, np.float32).reshape(np.asarray(in0).shape[0], -1)
    b = np.asarray(in1, np.float32).reshape(a.shape)
    return np.cumsum(a * b, axis=1, dtype=np.float32).reshape(
        np.asarray(in0).shape)


def _scan_muladd_ref(in0, in1, s0, s1, imm2):
    a = np.asarray(in0, np.float32).reshape(np.asarray(in0).shape[0], -1)
    b = np.asarray(in1, np.float32).reshape(a.shape)
    return np.cumsum(a * (b + s0), axis=1, dtype=np.float32).reshape(
        np.asarray(in0).shape)


def _u_edge_ref(in0, in1, s0, s1, imm2):
    a = np.asarray(in0, np.float32).reshape(np.asarray(in0).shape[0], -1)
    b = np.asarray(in1, np.float32).reshape(a.shape)
    e = np.cumsum(b + s0, axis=1, dtype=np.float32)
    return (a >= e).astype(np.float32).reshape(np.asarray(in0).shape)


def _register_ops():
    """Custom DVE ops:
    SCAN_MUL_ANT: out = chained_cumsum(in0*in1)  (masked-sum capture scans)
    U_EDGE_ANT:   out = (in0 >= chained_cumsum(in1 + s0))  (fused edges+cmp)
    """
    import concourse.dve_ops as dve_ops
    from concourse.dve_spec import Spec, Src0, Src1, C0, scan, AluOp, lower
    from concourse.dve_uop import DveOpSpec
    have = {op.name: op for op in dve_ops.OPS}
    specs = (
        ("SCAN_MUL_ANT", Spec(body=scan(AluOp.ADD, Src0 * Src1),
                              reference=_scan_mul_ref), True),
        ("SCAN_MULADD_ANT", Spec(body=scan(AluOp.ADD, Src0 * (Src1 + C0)),
                                 reference=_scan_muladd_ref), True),
        ("U_EDGE_ANT", Spec(body=Src0 >= scan(AluOp.ADD, Src1 + C0),
                            reference=_u_edge_ref), True),
    )
    out = []
    row = max(dve_ops._SUB_OPCODE_FOR_NAME.values()) + 1
    for name, spec, rd1 in specs:
        if name in have:
            out.append(have[name])
            continue
        assert row < 0x20
        shas = {}
        for ver in ("v3", "v4"):
            u = lower(spec, ver=ver)
            shas[ver] = DveOpSpec(name=name, opcode=row, uops=u,
                                  rd1_en=rd1).sha(ver)
        op = dve_ops.DveOp(name, spec, subdim=False, uops_sha=shas)
        dve_ops.OPS.append(op)
        dve_ops.CUSTOM_DVE_SPECS[name] = spec
        dve_ops._SUB_OPCODE_FOR_NAME[name] = row
        out.append(op)
        row += 1
    return out


class _Bacc(bacc.Bacc):
    """Bacc with a trimmed activation-table list so Exp and Ln share one
    table and Sigmoid another (no per-chunk ACT_TABLE_LOAD thrash)."""

    _KEEP_TABLES = ("natural_log_exp_and_others", "sigmoid_and_others")

    def insert_act_table_loads(self):
        import bass_rust as _bass_rust
        from concourse.hw_specs import get_activation_tables
        import concourse.mybir as _mb
        has_activation = any(
            isinstance(i, _mb.InstActivation)
            for b in self.main_func.blocks
            for i in b.instructions
        )
        if not has_activation:
            return
        all_tables = get_activation_tables(self.m.arch)
        tables = [(k, (v if k in self._KEEP_TABLES else set()))
                  for k, v in all_tables.items()]
        _bass_rust.insert_act_table_loads(self, tables)


def _build(bc, dbg=None):
    nch = bc // 128               # 16 row chunks of 128
    NBLK = 4
    BSW = bc // NBLK              # 512 rows per block
    GPB = BSW // 128              # 4 chunks per block
    scan_mul, scan_muladd, u_edge = _register_ops()
    nc = _Bacc("TRN2", target_bir_lowering=False, debug=False,
               enable_asserts=False)

    def din(name, shape, dt=F32):
        return nc.dram_tensor(name, list(shape), dt, kind="ExternalInput").ap()

    dbg_shapes = {
        "tbf": ([NBLK, 128, KH, bc // NBLK], BF16),
        "EW": ([bc // 128, 128, F, NB], F32),
        "EH": ([bc // 128, 128, F, NB], F32),
        "D": ([bc // 128, 128, F, NB + 1], BF16),
        "u": ([bc // 128, 128, F, NB], BF16),
        "cap": ([NBLK, 128, bc // NBLK // 128, 6, F], F32),
        "sm": ([NBLK, 128, 2, bc // NBLK // 128, F], F32),
        "xpc": ([bc // 128, 128, F], F32),
    }
    dbg_d = None
    if dbg is not None:
        shp, ddt = dbg_shapes[dbg]
        dbg_d = nc.dram_tensor("dbg", list(shp), ddt,
                               kind="ExternalOutput").ap()

    pred = din("pred", (bc, F))
    ctxm = din("ctx", (bc, C))
    win = din("win", (F, H), BF16)
    wc4 = din("wc4", (C, (NBLOCKS + 1) * H), BF16)
    wb1 = din("wb1", (NBLOCKS, H, H), BF16)
    wb2 = din("wb2", (NBLOCKS, H, H), BF16)
    wo_d = [din(f"wo{k}", (128, (NQ - k) * QCOLS), BF16) for k in range(KH)]
    b1 = din("b1", (H,))
    bb1 = din("bb1", (NBLOCKS, H))
    bb2 = din("bb2", (NBLOCKS, H))
    bcb = din("bcb", (NBLOCKS, H))
    ident = din("ident", (128, 128))
    iotaf = din("iotaf", (F,))
    out_d = nc.dram_tensor("out", [bc], F32, kind="ExternalOutput").ap()

    AX = mybir.AxisListType
    OP = mybir.AluOpType
    ACTF = mybir.ActivationFunctionType

    def bcast(apx, n):
        """AP -> same with a stride-0 inner dim of size n appended."""
        return bass.AP(tensor=apx.tensor, offset=apx.offset,
                       ap=list(apx.ap) + [[0, n]])

    def pbcast(ap1d, p):
        return bass.AP(tensor=ap1d.tensor, offset=ap1d.offset,
                       ap=[[0, p]] + list(ap1d.ap))

    with tile.TileContext(nc) as tc, ExitStack() as ctx:
        const = ctx.enter_context(tc.tile_pool(name="const", bufs=1))
        wp = ctx.enter_context(tc.tile_pool(name="wp", bufs=1))
        persist = ctx.enter_context(tc.tile_pool(name="persist", bufs=1))

        ident_t = const.tile([128, 128], F32)
        nc.sync.dma_start(out=ident_t[:], in_=ident)
        one_t = const.tile([128, 1], F32)
        nc.vector.memset(one_t[:], 1.0)
        mb_t = const.tile([128, 1], F32)
        nc.vector.memset(mb_t[:], MIN_BIN)
        md_t = const.tile([128, 1], F32)
        nc.vector.memset(md_t[:], MIN_DERIV)
        iota_t = const.tile([128, F], F32)
        nc.sync.dma_start(out=iota_t[:], in_=pbcast(iotaf, 128))

        # ---- persistent activations ----
        # ctx_T split in two so deferred in-loop transposes (chunks 8-15)
        # never write a tile the earlier blocks' trunk is reading.
        ctx_Ta = [persist.tile([128, bc // 2], BF16, tag=f"cTa{k}",
                               name=f"cTa{k}") for k in range(KH)]
        ctx_Tb = [persist.tile([128, bc // 2], BF16, tag=f"cTb{k}",
                               name=f"cTb{k}") for k in range(KH)]
        x_T = persist.tile([F, bc], BF16)
        prodb = persist.tile([128, nch], F32)

        def ctxT(k, sl):
            half = bc // 2
            if sl.stop <= half:
                return ctx_Ta[k][:, sl]
            return ctx_Tb[k][:, sl.start - half:sl.stop - half]

        # ---- weights (DMAs ordered so block 0 can start early) ----
        win_t = wp.tile([F, H], BF16)
        wc4_t = [wp.tile([128, (NBLOCKS + 1) * H], BF16, tag=f"wc4_{k}",
                         name=f"wc4_{k}") for k in range(KH)]
        wb1_t = [[wp.tile([128, H - 128 * k], BF16, tag=f"wb1_{i}_{k}",
                          name=f"wb1_{i}_{k}") for k in range(KH)]
                 for i in range(NBLOCKS)]
        wb2_t = [[wp.tile([128, H - 128 * k], BF16, tag=f"wb2_{i}_{k}",
                          name=f"wb2_{i}_{k}") for k in range(KH)]
                 for i in range(NBLOCKS)]
        wo_t = [wp.tile([128, (NQ - k) * QCOLS], BF16, tag=f"wo_{k}",
                        name=f"wo_{k}") for k in range(KH)]
        b1_t = wp.tile([128, KH], F32)
        bb1_t = wp.tile([128, NBLOCKS, KH], F32)
        bb2_t = wp.tile([128, NBLOCKS, KH], F32)
        bcb_t = wp.tile([128, NBLOCKS, KH], F32)

        # in-loop ctx-transpose psum/load pools (alive for the whole kernel)
        pst = ctx.enter_context(tc.tile_pool(name="pst", bufs=2, space="PSUM"))
        pld = ctx.enter_context(tc.tile_pool(name="pld", bufs=4))

        pending_ld = {}

        def ld_ctx(c):
            ld = pld.tile([128, C], F32, tag="ctxld", name="ctxld")
            for hh in range(2):
                nc.sync.dma_start(
                    out=ld[:, hh * (C // 2):(hh + 1) * (C // 2)],
                    in_=ctxm[c * 128:(c + 1) * 128,
                             hh * (C // 2):(hh + 1) * (C // 2)])
            pending_ld[c] = ld

        def tp_ctx(c):
            ld = pending_ld.pop(c, None)
            if ld is None:
                ld_ctx(c)
                ld = pending_ld.pop(c)
            ps = pst.tile([128, KH, 128], F32, tag="tp", name="tp")
            for k in range(KH):
                nc.tensor.transpose(ps[:, k, :], ld[:, k * 128:(k + 1) * 128],
                                    ident_t[:])
            half = bc // 2
            for k in range(KH):
                dstl = ctx_Ta[k] if c * 128 < half else ctx_Tb[k]
                o = c * 128 if c * 128 < half else c * 128 - half
                nc.scalar.activation(out=dstl[:, o:o + 128],
                                     in_=ps[:, k, :], func=ACTF.Copy)

        # prologue: block-0 inputs first (ctx+pred chunks 0-3), then weights
        # in first-use order (wo0 early: the first out-GEMM needs it right
        # after block 0's trunk), remaining transposes last.
        with tc.tile_pool(name="pstp", bufs=2, space="PSUM") as pstp:
            def tp_pred(c):
                pldp = pld.tile([128, F], F32, tag="predld", name="predld")
                nc.sync.dma_start(out=pldp[:],
                                  in_=pred[c * 128:(c + 1) * 128, :])
                ps = pstp.tile([F, 128], F32, tag="tpp", name="tpp")
                nc.tensor.transpose(ps[:], pldp[:], ident_t[:])
                nc.scalar.activation(out=x_T[:, c * 128:(c + 1) * 128],
                                     in_=ps[:], func=ACTF.Copy)

            for c in range(4):
                tp_ctx(c)
                tp_pred(c)
            for c in range(4, nch):
                tp_pred(c)
            nc.sync.dma_start(out=win_t[:], in_=win)
            W4 = (NBLOCKS + 1) * H
            for k in range(KH):
                for qq in range(4):
                    nc.sync.dma_start(
                        out=wc4_t[k][:, qq * (W4 // 4):(qq + 1) * (W4 // 4)],
                        in_=wc4[k * 128:(k + 1) * 128,
                                qq * (W4 // 4):(qq + 1) * (W4 // 4)])
            nc.sync.dma_start(out=b1_t[:], in_=b1.rearrange("(m p) -> p m",
                                                            p=128))
            for tt_, src in ((bb1_t, bb1), (bb2_t, bb2), (bcb_t, bcb)):
                nc.sync.dma_start(out=tt_[:],
                                  in_=src.rearrange("i (m p) -> p i m", p=128))
            def wo_piece(k, qq):
                hw_ = QCOLS // 2
                for hh in range(2):
                    a0 = qq * QCOLS + hh * hw_
                    nc.sync.dma_start(out=wo_t[k][:, a0:a0 + hw_],
                                      in_=wo_d[k][:, a0:a0 + hw_])

            for i in range(NBLOCKS):
                for k in range(KH):
                    ksl = slice(k * 128, (k + 1) * 128)
                    nc.sync.dma_start(out=wb1_t[i][k][:],
                                      in_=wb1[i, ksl, 128 * k:])
                    nc.sync.dma_start(out=wb2_t[i][k][:],
                                      in_=wb2[i, ksl, 128 * k:])
            # ctx 4-7 loads issued now (cheap, early); their transposes are
            # emitted AFTER block 0's trunk so the in-order PE queue never
            # stalls on them before t0
            for c in range(4, 8):
                ld_ctx(c)
            for q in range(NQ):
                for k in range(q + 1):
                    wo_piece(k, q - k)

        TS = nc.vector.tensor_scalar
        TT = nc.vector.tensor_tensor
        STT = nc.vector.scalar_tensor_tensor

        def tscopy(dst, srcap):
            TS(out=dst, in0=srcap, scalar1=0.0, scalar2=None, op0=OP.add)

        def flat3(t):
            return t[:].rearrange("p a b -> p (a b)")

        # ---- main pipeline pools ----
        with tc.tile_pool(name="psa", bufs=2, space="PSUM") as psa, \
             tc.tile_pool(name="psb", bufs=2, space="PSUM") as psb, \
             tc.tile_pool(name="blk", bufs=2) as blkp, \
             tc.tile_pool(name="blk1", bufs=1) as blkp1, \
             tc.tile_pool(name="spl", bufs=2) as spl, \
             tc.tile_pool(name="spl1", bufs=1) as spl1, \
             tc.tile_pool(name="win", bufs=1) as winp, \
             tc.tile_pool(name="chn", bufs=1) as chn:

            tbf_of = {}

            def trunk_steps(blk, dve_add=False):
                """Emit block blk's MADE trunk as 10 staggered closures,
                interleaved between the previous block's spline chunks so
                each op's cross-engine producers land a slot earlier (the
                per-engine queues are in-order: program order = schedule)."""
                bsl = slice(blk * BSW, (blk + 1) * BSW)
                st = {}

                def t0():
                    t_t = blkp.tile([128, KH, BSW], BF16, tag="t", name="t")
                    st["t"] = t_t
                    for m in range(KH):
                        msl = slice(m * 128, (m + 1) * 128)
                        ps = psa.tile([128, BSW], F32, tag="mma", name="mma")
                        nc.tensor.matmul(ps[:], win_t[:, msl], x_T[:, bsl],
                                         start=True, stop=False)
                        for k in range(KH):
                            nc.tensor.matmul(ps[:], wc4_t[k][:, msl],
                                             ctxT(k, bsl),
                                             start=False, stop=(k == KH - 1))
                        nc.scalar.activation(out=t_t[:, m, :], in_=ps[:],
                                             func=ACTF.Identity,
                                             bias=b1_t[:, m:m + 1])

                def gates(ii):
                    if ii == 0:
                        st["g"] = blkp1.tile([128, NBLOCKS, KH, BSW], BF16,
                                             tag="g", name="g")
                    g_t = st["g"]
                    for m in range(KH):
                        csl0 = (ii + 1) * H + m * 128
                        ps = psa.tile([128, BSW], F32, tag="mma", name="mma")
                        for k in range(KH):
                            nc.tensor.matmul(ps[:],
                                             wc4_t[k][:, csl0:csl0 + 128],
                                             ctxT(k, bsl),
                                             start=(k == 0), stop=(k == KH - 1))
                        nc.scalar.activation(out=g_t[:, ii, m, :], in_=ps[:],
                                             func=ACTF.Sigmoid,
                                             bias=bcb_t[:, ii, m:m + 1])

                def rb_a(ii):   # h1 relu + mm1 chains + h2 relus
                    h1 = blkp1.tile([128, KH, BSW], BF16, tag="h1", name="h1")
                    nc.scalar.activation(out=flat3(h1), in_=flat3(st["t"]),
                                         func=ACTF.Relu)
                    h2 = blkp1.tile([128, KH, BSW], BF16, tag="h2", name="h2")
                    st["h2"] = h2
                    for m in range(KH):
                        ps = psa.tile([128, BSW], F32, tag="mma", name="mma")
                        for k in range(m + 1):
                            off = (m - k) * 128
                            nc.tensor.matmul(ps[:],
                                             wb1_t[ii][k][:, off:off + 128],
                                             h1[:, k, :],
                                             start=(k == 0), stop=(k == m))
                        nc.scalar.activation(out=h2[:, m, :], in_=ps[:],
                                             func=ACTF.Relu,
                                             bias=bb1_t[:, ii, m:m + 1])

                def rb_b(ii):   # mm2 chains + gated v = (psum+bb2)*g
                    h2 = st["h2"]
                    v = blkp1.tile([128, KH, BSW], BF16, tag="v", name="v")
                    st["v"] = v
                    for m in range(KH):
                        ps2 = psa.tile([128, BSW], F32, tag="mma", name="mma")
                        for k in range(m + 1):
                            off = (m - k) * 128
                            nc.tensor.matmul(ps2[:],
                                             wb2_t[ii][k][:, off:off + 128],
                                             h2[:, k, :],
                                             start=(k == 0), stop=(k == m))
                        STT(out=v[:, m, :], in0=ps2[:],
                            scalar=bb2_t[:, ii, m:m + 1],
                            in1=st["g"][:, ii, m, :], op0=OP.add, op1=OP.mult)

                def rb_fin(ii):  # t += v (gp flat add; DVE ping-pong when
                    # the DVE is otherwise idle, i.e. block 0 at startup)
                    if dve_add:
                        t2 = blkp.tile([128, KH, BSW], BF16, tag="t", name="t")
                        TT(out=t2[:], in0=st["t"][:], in1=st["v"][:],
                           op=OP.add)
                        st["t"] = t2
                    else:
                        nc.gpsimd.tensor_tensor(out=st["t"][:],
                                                in0=st["t"][:],
                                                in1=st["v"][:], op=OP.add)

                def tbf_cp():
                    # t is bf16: the out-GEMM reads it directly, no copy
                    tbf_of[blk] = st["t"]
                    if dbg == "tbf":
                        nc.sync.dma_start(out=dbg_d[blk], in_=st["t"][:])

                return [
                    t0,                                   # s0 (pre-a0)
                    lambda: gates(0),                     # s1
                    lambda: (gates(1), rb_a(0)),          # s2
                    lambda: (gates(2), rb_b(0)),          # s3
                    lambda: (rb_fin(0), rb_a(1)),         # s4
                    lambda: rb_b(1),                      # s5
                    lambda: (rb_fin(1), rb_a(2)),         # s6
                    lambda: (rb_b(2), rb_fin(2)),         # s7
                    lambda: None,                         # s8
                    tbf_cp,                               # s9 (post-chain)
                ]

            # block 0's trunk runs un-overlapped (nothing to hide it behind)
            for p in trunk_steps(0, dve_add=True):
                p()
            for c in range(4, 8):
                tp_ctx(c)

            for blk in range(NBLK):
                tbf = tbf_of.pop(blk)
                # ---- per-block window buffers ----
                capb = winp.tile([128, GPB, 6, F], F32, tag="capb",
                                 name="capb")
                extb = winp.tile([128, GPB, 4, F], BF16, tag="extb",
                                 name="extb")
                xb = winp.tile([128, GPB, F], F32, tag="xb", name="xb")
                xpcw = winp.tile([128, GPB, F], F32, tag="xpcw", name="xpcw")

                def stage_a(gi):
                    c = blk * GPB + gi
                    csl = slice(c * 128, (c + 1) * 128)
                    gsl = slice(gi * 128, (gi + 1) * 128)
                    nc.sync.dma_start(out=xb[:, gi, :], in_=pred[csl, :])

                    EW = spl1.tile([128, F, NB], BF16, tag="EW", name="EW")
                    EH = spl1.tile([128, F, NB], BF16, tag="EH", name="EH")
                    # Dt is consumed within stage_a (dD + extracts): 1 buf.
                    Dt = spl1.tile([128, F, NB + 1], BF16, tag="Dt", name="Dt")
                    for ty in range(3):       # 0=w 1=h 2=d
                        for pa, pair in enumerate(PAIRS):
                            ncols = DQ if ty == 2 else WQ
                            ps = psb.tile([128, 2, 512], F32, tag="po",
                                          name="po")
                            for si, q in enumerate(pair):
                                for k in range(q + 1):
                                    off = ((q - k) * QCOLS + ty * WQ)
                                    nc.tensor.matmul(
                                        ps[:, si, 0:ncols],
                                        tbf[:, k, gsl],
                                        wo_t[k][:, off:off + ncols],
                                        start=(k == 0), stop=(k == q))
                            qa, qb = pair
                            dst = Dt if ty == 2 else (EW if ty == 0 else EH)
                            dstp = dst[:]
                            out2 = bass.AP(
                                tensor=dstp.tensor,
                                offset=dstp.offset + qa * ncols,
                                ap=[dstp.ap[0], [(qb - qa) * ncols, 2],
                                    [1, ncols]])
                            nc.scalar.activation(
                                out=out2, in_=ps[:, :, 0:ncols],
                                func=ACTF.Exp,
                                scale=(1.0 if ty == 2 else SCALE))
                    # D = ln(exp(ud) + 1) in place
                    nc.scalar.activation(
                        out=Dt[:].rearrange("p f n -> p (f n)"),
                        in_=Dt[:].rearrange("p f n -> p (f n)"),
                        func=ACTF.Ln, bias=one_t[:])
                    # per-feature sums and normalizers.  W-side reciprocal
                    # must be accurate: the chained-edge trick needs
                    # sum(EWn) + NB*MIN_BIN == 1 to ~1e-6.
                    swf = spl1.tile([128, F], F32, tag="swf", name="swf")
                    shf = spl1.tile([128, F], F32, tag="shf", name="shf")
                    nc.vector.tensor_reduce(out=swf[:], in_=EW[:],
                                            axis=AX.X, op=OP.add)
                    nc.vector.tensor_reduce(out=shf[:], in_=EH[:],
                                            axis=AX.X, op=OP.add)
                    rsq = spl1.tile([128, 2, F], F32, tag="rsq", name="rsq")
                    nc.vector.reciprocal(out=rsq[:, 0, :], in_=swf[:])
                    nc.vector.reciprocal_approx_fast(out=rsq[:, 1, :],
                                                     in_=shf[:])
                    crs = spl1.tile([128, 2, F], F32, tag="crs", name="crs")
                    TS(out=flat3(crs), in0=flat3(rsq), scalar1=CF,
                       scalar2=None, op0=OP.mult)
                    # normalized W stream (fp32, for the fused edge compare
                    # only; the masked scans read the raw streams)
                    EWn = spl1.tile([128, F, NB], F32, tag="EWn", name="EWn")
                    TT(out=EWn[:], in0=EW[:],
                       in1=bcast(crs[:, 0, :], NB), op=OP.mult)
                    EHn = spl1.tile([128, F, NB], BF16, tag="EHn", name="EHn")
                    TT(out=EHn[:], in0=EH[:],
                       in1=bcast(crs[:, 1, :], NB), op=OP.mult)
                    # chained compare operand: x + f
                    TT(out=xpcw[:, gi, :], in0=xb[:, gi, :], in1=iota_t[:],
                       op=OP.add)
                    # fused edges+compare: u[j] = [x+f >= e_{j+1}], chained
                    u30 = spl.tile([128, F, NB], BF16, tag="u30", name="u30")
                    nc.vector._custom_dve(u_edge, out=u30[:],
                                          in0=bcast(xpcw[:, gi, :], NB),
                                          in1=EWn[:], s0=MIN_BIN)
                    # W/H masked scans on the normalized streams with the
                    # +MIN_BIN term folded in: captures ARE incw/inch sums
                    for s in range(4):
                        srct = EWn if s < 2 else EHn
                        cap = bass.AP(
                            tensor=capb[:].tensor,
                            offset=capb[:].offset + (gi * 6 + s) * F,
                            ap=[capb[:].ap[0], [1, F], [0, NB - 1]])
                        ssl = bass.AP(tensor=srct[:].tensor,
                                      offset=srct[:].offset + (s % 2),
                                      ap=[srct[:].ap[0], [NB, F], [1, NB - 1]])
                        nc.vector._custom_dve(scan_muladd, out=cap,
                                              in0=u30[:, :, 0:NB - 1],
                                              in1=ssl, s0=MIN_BIN)
                    # softplus differences on DVE: ANY sustained GpSimd
                    # activity during the spline taxes every engine 25-40%
                    # (SBUF contention), so this stays here despite gp slack
                    dD = spl.tile([128, F, NB], BF16, tag="dD", name="dD")
                    TT(out=dD[:], in0=Dt[:, :, 1:NB + 1],
                       in1=Dt[:, :, 0:NB], op=OP.subtract)
                    # extracts: ew0/eh0 normalized (+MIN_BIN folded);
                    # d0/d1 with +MIN_DERIV folded in
                    for j, (src, st_, o, bia) in enumerate((
                            (EWn[:], NB, 0, mb_t[:]), (EHn[:], NB, 0, mb_t[:]),
                            (Dt[:], NB + 1, 0, md_t[:]),
                            (Dt[:], NB + 1, 1, md_t[:]))):
                        src_ap = bass.AP(tensor=src.tensor,
                                         offset=src.offset + o,
                                         ap=[src.ap[0], [st_, F]])
                        if bia is None:
                            nc.scalar.activation(out=extb[:, gi, j, :],
                                                 in_=src_ap, func=ACTF.Copy)
                        else:
                            nc.scalar.activation(out=extb[:, gi, j, :],
                                                 in_=src_ap,
                                                 func=ACTF.Identity, bias=bia)
                    if dbg == "EW":
                        nc.sync.dma_start(out=dbg_d[c], in_=EWn[:])
                    if dbg == "EH":
                        nc.sync.dma_start(out=dbg_d[c], in_=EH[:])
                    if dbg == "D":
                        nc.sync.dma_start(out=dbg_d[c], in_=Dt[:])
                    if dbg == "xpc":
                        nc.sync.dma_start(out=dbg_d[c], in_=xpcw[:, gi, :])
                    if dbg == "u":
                        nc.sync.dma_start(out=dbg_d[c], in_=u30[:])
                    return dD, u30

                def stage_b(gi, tiles):
                    dD, u30 = tiles
                    # D-side masked-sum scans (deferred a stage: dD comes
                    # from GpSimd and must not stall the DVE queue)
                    for s in range(2):
                        cap = bass.AP(
                            tensor=capb[:].tensor,
                            offset=capb[:].offset + (gi * 6 + s + 4) * F,
                            ap=[capb[:].ap[0], [1, F], [0, NB - 1]])
                        dsl = bass.AP(tensor=dD[:].tensor,
                                      offset=dD[:].offset + s,
                                      ap=[dD[:].ap[0], [NB, F], [1, NB - 1]])
                        nc.vector._custom_dve(scan_mul, out=cap,
                                              in0=u30[:, :, 0:NB - 1],
                                              in1=dsl)

                # deferred ctx transposes: chunks 8-15 during blocks 0-1
                tp_sched = {0: [8, 9, 10, 11], 1: [12, 13, 14, 15]}
                tps = tp_sched.get(blk, [])
                steps = trunk_steps(blk + 1) if blk + 1 < NBLK else []

                def filler(sl):
                    if 0 <= sl - 1 < len(tps):
                        tp_ctx(tps[sl - 1])
                    if sl + 1 < len(tps):
                        ld_ctx(tps[sl + 1])
                    if sl < len(steps):
                        steps[sl]()

                # slot 0 (t0 of the next trunk) goes BEFORE stage_a(0): the
                # in-order PE queue must not park it behind this block's
                # first out-GEMM (which waits on tbf/exp consumers).
                for c0 in tps[:2]:
                    ld_ctx(c0)
                filler(0)
                pend = stage_a(0)
                filler(1)
                nxt = stage_a(1)
                filler(2)
                stage_b(0, pend)
                filler(3)
                pend = nxt
                nxt = stage_a(2)
                filler(4)
                stage_b(1, pend)
                filler(5)
                pend = nxt
                nxt = stage_a(3)
                filler(6)
                stage_b(2, pend)
                filler(7)
                stage_b(3, nxt)
                filler(8)

                if dbg == "cap":
                    nc.sync.dma_start(out=dbg_d[blk], in_=capb[:])
                if dbg == "sm":
                    nc.sync.dma_start(out=dbg_d[blk, :, 1], in_=extb[:, :, 0])

                # ---- deferred per-feature chain, [128, GPB*F] tiles ----
                def ct(nm):
                    return chn.tile([128, GPB, F], F32, tag=nm, name=nm)

                def flat(t):
                    return t[:].rearrange("p a f -> p (a f)")

                # un-chain captures: gd[f] = cap[f] - cap[f-1] within each gi
                gd = chn.tile([128, GPB, 6, F], F32, tag="gd", name="gd")
                tscopy(gd[:, :, :, 0:1], capb[:, :, :, 0:1])
                TT(out=gd[:, :, :, 1:F], in0=capb[:, :, :, 1:F],
                   in1=capb[:, :, :, 0:F - 1], op=OP.subtract)
                q1 = gd[:, :, 0, :]
                q2 = gd[:, :, 1, :]
                r1 = gd[:, :, 2, :]
                r2 = gd[:, :, 3, :]
                g5 = gd[:, :, 4, :]
                g6 = gd[:, :, 5, :]
                ew0b = extb[:, :, 0, :]
                eh0b = extb[:, :, 1, :]
                d0b = extb[:, :, 2, :]
                d1b = extb[:, :, 3, :]

                w1 = ct("w1"); w2 = ct("w2"); w3 = ct("w3")
                w6 = ct("w6"); w7 = ct("w7"); w8 = ct("w8")
                w9 = xb  # xb is dead after th; reuse
                # q1 IS incw (scans ran on the normalized stream with +MB);
                # w2 = inw = (q2 - q1) + (EWn[0]+MB) -> rw
                TT(out=w2[:], in0=q2, in1=q1, op=OP.subtract)
                TT(out=w2[:], in0=w2[:], in1=ew0b, op=OP.add)
                nc.vector.reciprocal_approx_fast(out=flat(w2), in_=flat(w2))
                # w3 = th = (x - incw) * rw
                TT(out=w3[:], in0=xb[:], in1=q1, op=OP.subtract)
                TT(out=w3[:], in0=w3[:], in1=w2[:], op=OP.mult)
                # w6 = inh = (r2 - r1) + (EHn[0]+MB)
                TT(out=w6[:], in0=r2, in1=r1, op=OP.subtract)
                TT(out=w6[:], in0=w6[:], in1=eh0b, op=OP.add)
                # w1 = ind ; w8 = s2 = ind + indp1
                TT(out=w1[:], in0=g5, in1=d0b, op=OP.add)
                TT(out=w8[:], in0=g6, in1=d1b, op=OP.add)
                TT(out=w8[:], in0=w8[:], in1=w1[:], op=OP.add)
                # w7 = tt = th*(1-th) ; w9 = th^2
                nc.scalar.activation(out=flat(w7), in_=flat(w3),
                                     func=ACTF.Identity, bias=one_t[:],
                                     scale=-1.0)
                TT(out=w7[:], in0=w3[:], in1=w7[:], op=OP.mult)
                nc.scalar.activation(out=flat(w9), in_=flat(w3),
                                     func=ACTF.Square)
                # w2 = dl = inh * rw (rw dead)
                TT(out=w2[:], in0=w6[:], in1=w2[:], op=OP.mult)
                # w9 = num = inh*(dl*th^2 + ind*tt)
                TT(out=w9[:], in0=w9[:], in1=w2[:], op=OP.mult)
                TT(out=w1[:], in0=w1[:], in1=w7[:], op=OP.mult)
                TT(out=w9[:], in0=w9[:], in1=w1[:], op=OP.add)
                TT(out=w9[:], in0=w9[:], in1=w6[:], op=OP.mult)
                # w8 = den = dl + (s2 - 2dl)*tt -> rden
                STT(out=w8[:], in0=w2[:], scalar=-2.0, in1=w8[:],
                    op0=OP.mult, op1=OP.add)
                TT(out=w8[:], in0=w8[:], in1=w7[:], op=OP.mult)
                TT(out=w8[:], in0=w8[:], in1=w2[:], op=OP.add)
                nc.vector.reciprocal_approx_fast(out=flat(w8), in_=flat(w8))
                sh = r1  # r1 IS inch (normalized captures)
                cdf = w9
                TT(out=cdf[:], in0=cdf[:], in1=w8[:], op=OP.mult)
                TT(out=cdf[:], in0=cdf[:], in1=sh, op=OP.add)
                # product over 64 features
                TT(out=cdf[:, :, 0:32], in0=cdf[:, :, 0:32],
                   in1=cdf[:, :, 32:64], op=OP.mult)
                TT(out=cdf[:, :, 0:16], in0=cdf[:, :, 0:16],
                   in1=cdf[:, :, 16:32], op=OP.mult)
                TT(out=cdf[:, :, 0:8], in0=cdf[:, :, 0:8],
                   in1=cdf[:, :, 8:16], op=OP.mult)
                TT(out=cdf[:, :, 0:4], in0=cdf[:, :, 0:4],
                   in1=cdf[:, :, 4:8], op=OP.mult)
                TT(out=cdf[:, :, 0:2], in0=cdf[:, :, 0:2],
                   in1=cdf[:, :, 2:4], op=OP.mult)
                TT(out=prodb[:, blk * GPB:(blk + 1) * GPB],
                   in0=cdf[:, :, 0:1].rearrange("p a f -> p (a f)"),
                   in1=cdf[:, :, 1:2].rearrange("p a f -> p (a f)"),
                   op=OP.mult)
                # next block's tbf copy: emitted after the chain so the
                # in-order DVE queue never waits on the trunk's last gp add
                filler(9)

            nc.sync.dma_start(out=out_d.rearrange("(c p) -> p c", p=128),
                              in_=prodb[:])

    nc.compile()
    return nc


def _to_bf16(a):
    import ml_dtypes
    return np.ascontiguousarray(np.asarray(a, np.float32)).astype(
        ml_dtypes.bfloat16)


def _prep_shared(W_in, b_in, Wc_in, bc_in, Wb1, bb1, Wb2, bb2, Wcb, bcb,
                 W_out, b_out):
    m_in, m_hh, m_out, d_h = _masks()
    assert not np.any(np.asarray(b_out)), "nonzero b_out not supported"
    perm = np.argsort(d_h, kind="stable")

    W_in = np.asarray(W_in, np.float32) * m_in
    W_in = W_in[:, perm]
    Wc_in = np.asarray(Wc_in, np.float32)[:, perm]
    Wcb = np.asarray(Wcb, np.float32)[:, :, perm]
    Wb1p = (np.asarray(Wb1, np.float32) * m_hh[None])[:, perm][:, :, perm]
    Wb2p = (np.asarray(Wb2, np.float32) * m_hh[None])[:, perm][:, :, perm]
    W_outp = (np.asarray(W_out, np.float32) * m_out)[perm, :]
    b1 = (np.asarray(b_in, np.float32) + np.asarray(bc_in, np.float32))[perm]
    bb1p = np.asarray(bb1, np.float32)[:, perm]
    bb2p = np.asarray(bb2, np.float32)[:, perm]
    bcbp = np.asarray(bcb, np.float32)[:, perm]

    wc4 = np.concatenate([Wc_in, Wcb[0], Wcb[1], Wcb[2]], axis=1)

    # packed W_out blobs: chunk k holds quarters q>=k as [w(480)|h(480)|d(496)]
    wo = []
    for k in range(KH):
        cols = []
        for q in range(k, NQ):
            for ty in range(3):
                for f in range(FQ * q, FQ * (q + 1)):
                    if ty == 0:
                        cols.extend(range(f * MULT, f * MULT + NB))
                    elif ty == 1:
                        cols.extend(range(f * MULT + NB, f * MULT + 2 * NB))
                    else:
                        cols.extend(range(f * MULT + 2 * NB, (f + 1) * MULT))
        blob = W_outp[k * 128:(k + 1) * 128][:, cols]
        assert blob.shape == (128, (NQ - k) * QCOLS)
        wo.append(_to_bf16(blob))

    shared = {
        "win": _to_bf16(W_in),
        "wc4": _to_bf16(wc4),
        "wb1": _to_bf16(Wb1p),
        "wb2": _to_bf16(Wb2p),
        "b1": np.ascontiguousarray(b1),
        "bb1": np.ascontiguousarray(bb1p),
        "bb2": np.ascontiguousarray(bb2p),
        "bcb": np.ascontiguousarray(bcbp),
        "ident": np.eye(128, dtype=np.float32),
        "iotaf": np.arange(F, dtype=np.float32),
    }
    for k in range(KH):
        shared[f"wo{k}"] = wo[k]
    return shared


def kernel(predicates, contexts, W_in, b_in, Wc_in, bc_in, Wb1, bb1, Wb2, bb2,
           Wcb, bcb, W_out, b_out):
    global LAST_RESULTS
    predicates = np.asarray(predicates, dtype=np.float32)
    contexts = np.asarray(contexts, dtype=np.float32)
    bc = predicates.shape[0] // NCORES
    key = (bc, DBG)
    if key not in _CACHE:
        _CACHE[key] = _build(bc, dbg=DBG)
    nc = _CACHE[key]
    shared = _prep_shared(W_in, b_in, Wc_in, bc_in, Wb1, bb1, Wb2, bb2,
                          Wcb, bcb, W_out, b_out)
    in_maps = []
    for cid in range(NCORES):
        sl = slice(cid * bc, (cid + 1) * bc)
        m = dict(shared)
        m["pred"] = np.ascontiguousarray(predicates[sl])
        m["ctx"] = np.ascontiguousarray(contexts[sl])
        in_maps.append(m)
    res = run_bass_kernel_spmd(nc, in_maps, core_ids=list(range(NCORES)),
                               trace=TRACE)
    LAST_RESULTS = res
    return np.concatenate([res.results[i]["out"] for i in range(NCORES)])
